# revision 1
# baseline (speedup 1.0000x reference)
"""Multi-head attention (RoPE) Trainium2 kernel, 8-way sharded.

Sharding: core c handles batch b = c//4 and 4 heads h0 = 4*(c%4).
Per-core device program (all layouts chosen so no on-device transposes
are needed; host pre-transposes/slices the inputs, in fp16):

  inputs (per core):
    xT   [1024, 2048]  = x[b].T                          (fp16)
    wqkT [1024, 512]   = concat(w_q_rows, w_k_rows).T    (fp16)
    wvT  [1024, 256]   = w_v_rows.T                      (fp16)
    woT  [256, 1024]   = w_out[:, head_cols].T           (fp16)
    cosT [128, 2048]   rope cos table, row r -> dim r%64 (2 heads stacked)
    sinT [128, 2048]   rope sin table with rotate-half sign baked in

  program:
    qkT  = (wqkT.T @ xT tiles) + rope        [512, 2048]  (feat-major, fp16)
    V'   = x @ w_v.T  (+ ones col per head)  [2048, 4*65] (token-major, fp16)
    per head: S.T[kt,qt] = k'T.T @ q'T ; P = exp(0.125*S) (no max; |S|<9)
              outT[65,qt] = [V|1].T @ P  (row 64 = softmax denominator)
              attn_outT = outT[:64] * bcast(recip_approx(outT[64]))
    y = attn_out @ w_out_slice.T             [2048, 1024] (fp32 partial)

  host: y[b] = sum of the 4 per-core partials.
"""

import numpy as np

B = 2
N = 2048
C = 1024
H_TOT = 16
HD = 64
HC = 4  # heads per core
N_CORES = 8
ROPE_BASE = 10000.0

_PROGRAM = None


def _rope_tables():
    inv_freq = 1.0 / (ROPE_BASE ** (np.arange(0, HD, 2, dtype=np.float32) / HD))
    t = np.arange(N, dtype=np.float32)
    freqs = np.einsum("i,j->ij", t, inv_freq).astype(np.float32)  # [N, 32]
    emb = np.concatenate([freqs, freqs], axis=-1)  # [N, 64]
    cos = np.cos(emb).astype(np.float32)
    sin = np.sin(emb).astype(np.float32)
    cosT = np.ascontiguousarray(np.tile(cos.T, (2, 1)))  # [128, 2048]
    sinT = sin.T.copy()  # [64, 2048]
    sinT_signed = np.concatenate([-sinT[:32], sinT[32:]], axis=0)  # sign for rot-half
    sinT2 = np.ascontiguousarray(np.tile(sinT_signed, (2, 1)))  # [128, 2048]
    return cosT, sinT2


def _build_program(debug=False):
    import concourse.mybir as mybir
    import concourse.tile as tile
    from concourse import bacc

    f32 = mybir.dt.float32
    f16 = mybir.dt.float16
    MUL = mybir.AluOpType.mult
    ADD = mybir.AluOpType.add
    EXP = mybir.ActivationFunctionType.Exp

    nc = bacc.Bacc("TRN2", target_bir_lowering=False, debug=False, num_devices=N_CORES)

    xT_d = nc.dram_tensor("xT", [C, N], f16, kind="ExternalInput").ap()
    wqk_d = nc.dram_tensor("wqkT", [C, 2 * HC * HD], f16, kind="ExternalInput").ap()
    wv_d = nc.dram_tensor("wvT", [C, HC * HD], f16, kind="ExternalInput").ap()
    wo_d = nc.dram_tensor("woT", [HC * HD, C], f16, kind="ExternalInput").ap()
    cos_d = nc.dram_tensor("cosT", [128, N], f32, kind="ExternalInput").ap()
    sin_d = nc.dram_tensor("sinT", [128, N], f32, kind="ExternalInput").ap()
    y_d = nc.dram_tensor("y", [N, C], f32, kind="ExternalOutput").ap()
    if debug:
        qk_dbg = nc.dram_tensor("qk_dbg", [4, 128, N], f16, kind="ExternalOutput").ap()
        vv_dbg = nc.dram_tensor(
            "vv_dbg", [128, 16 * HC * (HD + 1)], f16, kind="ExternalOutput"
        ).ap()
        es_dbg = nc.dram_tensor("es_dbg", [128, N], f16, kind="ExternalOutput").ap()
        nb_dbg = nc.dram_tensor("nb_dbg", [HD + 1, N], f32, kind="ExternalOutput").ap()
        ao_dbg = nc.dram_tensor("ao_dbg", [2, 128, N], f16, kind="ExternalOutput").ap()

    with tile.TileContext(nc) as tc:
        with (
            tc.tile_pool(name="persist", bufs=1) as persist,
            tc.tile_pool(name="work", bufs=2) as work,
            tc.tile_pool(name="psum", bufs=2, space="PSUM") as psp,
        ):
            qk = [
                persist.tile([128, N], f16, tag=f"qk{i}", name=f"qk{i}")
                for i in range(4)
            ]
            vv = persist.tile([128, 16, HC, HD + 1], f16, tag="vv", name="vv")
            ao = [
                persist.tile([128, N], f16, tag=f"ao{i}", name=f"ao{i}")
                for i in range(2)
            ]
            xT = persist.tile([128, 8, N], f16, tag="xT", name="xT")
            wqk = persist.tile([128, 8, 2 * HC * HD], f16, tag="wqk", name="wqk")
            wv = persist.tile([128, 8, HC * HD], f16, tag="wv", name="wv")
            wo = persist.tile([128, 2, C], f16, tag="wo", name="wo")
            cosT = persist.tile([128, N], f32, tag="cosT", name="cosT")
            sinT = persist.tile([128, N], f32, tag="sinT", name="sinT")

            nc.sync.dma_start(cosT[:], cos_d[:, :])
            nc.sync.dma_start(sinT[:], sin_d[:, :])
            for i in range(8):
                nc.sync.dma_start(xT[:, i, :], xT_d[i * 128 : (i + 1) * 128, :])
                nc.sync.dma_start(wqk[:, i, :], wqk_d[i * 128 : (i + 1) * 128, :])
                nc.sync.dma_start(wv[:, i, :], wv_d[i * 128 : (i + 1) * 128, :])
            for i in range(2):
                nc.sync.dma_start(wo[:, i, :], wo_d[i * 128 : (i + 1) * 128, :])

            def qk_proj(pt):
                """q/k projection + rope for one 128-feat tile (feat-major)."""
                bp = psp.tile([128, N], f32, tag="big", name=f"qkps{pt}")
                for tck in range(4):
                    sl = slice(tck * 512, (tck + 1) * 512)
                    for ct in range(8):
                        nc.tensor.matmul(
                            bp[:, sl],
                            wqk[:, ct, pt * 128 : (pt + 1) * 128],
                            xT[:, ct, sl],
                            start=(ct == 0),
                            stop=(ct == 7),
                        )
                for tck in range(4):
                    sl = slice(tck * 512, (tck + 1) * 512)
                    t_sb = work.tile([128, 512], f32, tag="ropet", name="rt")
                    u_sb = work.tile([128, 512], f32, tag="ropeu", name="ru")
                    nc.vector.tensor_tensor(t_sb[:], bp[:, sl], cosT[:, sl], MUL)
                    for o_lo, i_lo in [(0, 32), (32, 0), (64, 96), (96, 64)]:
                        nc.vector.tensor_tensor(
                            u_sb[o_lo : o_lo + 32, :],
                            bp[i_lo : i_lo + 32, sl],
                            sinT[o_lo : o_lo + 32, sl],
                            MUL,
                        )
                    nc.vector.tensor_tensor(qk[pt][:, sl], t_sb[:], u_sb[:], ADD)

            def v_proj(tt):
                """V' tile for one 128-token block (token-major) + ones col."""
                nc.vector.memset(vv[:, tt, :, HD], 1.0)
                vp = psp.tile([128, HC * HD], f32, tag="big", name=f"vps{tt}")
                for ct in range(8):
                    nc.tensor.matmul(
                        vp[:, :],
                        xT[:, ct, tt * 128 : (tt + 1) * 128],
                        wv[:, ct, :],
                        start=(ct == 0),
                        stop=(ct == 7),
                    )
                nc.vector.tensor_copy(
                    vv[:, tt, :, 0:HD],
                    vp[:].rearrange("p (h d) -> p h d", h=HC),
                )

            def attention(h):
                qpt = h // 2
                roff = (h % 2) * 64
                pv = psp.tile([HD + 1, N], f32, tag="big", name=f"pv{h}")
                for kt in range(16):
                    sp = psp.tile([128, N], f32, tag="big", name=f"sps{h}_{kt}")
                    for qc in range(4):
                        sl = slice(qc * 512, (qc + 1) * 512)
                        nc.tensor.matmul(
                            sp[:, sl],
                            qk[2 + qpt][roff : roff + 64, kt * 128 : (kt + 1) * 128],
                            qk[qpt][roff : roff + 64, sl],
                            start=True,
                            stop=True,
                        )
                    es = work.tile([128, N], f16, tag="es", bufs=3, name="es")
                    nc.scalar.activation(es[:], sp[:], EXP, scale=float(HD**-0.5))
                    if debug and h == 0 and kt == 0:
                        nc.sync.dma_start(es_dbg[:, :], es[:])
                    for qc in range(4):
                        sl = slice(qc * 512, (qc + 1) * 512)
                        nc.tensor.matmul(
                            pv[:, sl],
                            vv[:, kt, h, :],
                            es[:, sl],
                            start=(kt == 0),
                            stop=(kt == 15),
                        )
                rr = work.tile([1, N], f32, tag="rr", name="rr")
                ra = work.tile([1, N], f32, tag="ra", name="ra")
                nb = work.tile([HD, N], f32, tag="nb", name="nb")
                nc.vector.tensor_copy(rr[0:1, :], pv[HD : HD + 1, :])
                nc.vector.reciprocal_approx_fast(ra[0:1, :], rr[0:1, :])
                nc.gpsimd.partition_broadcast(nb[0:HD, :], ra[0:1, :])
                nc.vector.tensor_tensor(
                    ao[qpt][roff : roff + 64, :], pv[0:HD, :], nb[0:HD, :], MUL
                )
                if debug and h == 0:
                    nc.sync.dma_start(nb_dbg[0:HD, :], nb[:])
                    nc.sync.dma_start(nb_dbg[HD : HD + 1, :], ra[:])

            def out_proj(tt):
                yps = psp.tile([128, C], f32, tag="big", name=f"yps{tt}")
                for oc in range(2):
                    osl = slice(oc * 512, (oc + 1) * 512)
                    for ft in range(2):
                        nc.tensor.matmul(
                            yps[:, osl],
                            ao[ft][:, tt * 128 : (tt + 1) * 128],
                            wo[:, ft, osl],
                            start=(ft == 0),
                            stop=(ft == 1),
                        )
                ysb = work.tile([128, C], f32, tag="y", bufs=3, name="ysb")
                nc.vector.tensor_copy(ysb[:], yps[:])
                nc.sync.dma_start(y_d[tt * 128 : (tt + 1) * 128, :], ysb[:])

            # emission order chosen so attention on heads 0/1 can start
            # while q/k projection of heads 2/3 still runs
            qk_proj(0)  # q heads 0,1
            qk_proj(2)  # k heads 0,1
            for tt in range(16):
                v_proj(tt)
            attention(0)
            qk_proj(1)  # q heads 2,3
            qk_proj(3)  # k heads 2,3
            attention(1)
            if debug:
                for pt in range(4):
                    nc.sync.dma_start(qk_dbg[pt], qk[pt][:])
                nc.sync.dma_start(vv_dbg[:, :], vv[:].rearrange("p a b c -> p (a b c)"))
            attention(2)
            attention(3)
            for tt in range(16):
                out_proj(tt)
            if debug:
                for i in range(2):
                    nc.sync.dma_start(ao_dbg[i], ao[i][:])

    nc.compile()
    return nc


def _get_program():
    global _PROGRAM
    if _PROGRAM is None:
        _PROGRAM = _build_program()
    return _PROGRAM


def _make_in_maps(x, w_qkv, w_out):
    x = np.asarray(x, dtype=np.float32)
    w_qkv = np.asarray(w_qkv, dtype=np.float32)
    w_out = np.asarray(w_out, dtype=np.float32)
    cosT, sinT = _rope_tables()
    in_maps = []
    for c in range(N_CORES):
        b = c // 4
        h0 = HC * (c % 4)
        rows = np.arange(h0 * HD, (h0 + HC) * HD)
        wq = w_qkv[rows]  # [256, 1024]
        wk = w_qkv[C + rows]
        wv = w_qkv[2 * C + rows]
        in_maps.append(
            {
                "xT": np.ascontiguousarray(x[b].T).astype(np.float16),
                "wqkT": np.ascontiguousarray(np.concatenate([wq, wk], 0).T).astype(
                    np.float16
                ),
                "wvT": np.ascontiguousarray(wv.T).astype(np.float16),
                "woT": np.ascontiguousarray(w_out[:, rows].T).astype(np.float16),
                "cosT": cosT,
                "sinT": sinT,
            }
        )
    return in_maps


def run(inputs, trace=False, trace_cores=None):
    from concourse.bass_utils import run_bass_kernel_spmd

    nc = _get_program()
    in_maps = _make_in_maps(inputs["x"], inputs["w_qkv"], inputs["w_out"])
    res = run_bass_kernel_spmd(
        nc,
        in_maps,
        core_ids=list(range(N_CORES)),
        trace=trace,
        trace_cores=trace_cores,
    )
    y = np.zeros((B, N, C), dtype=np.float32)
    for c in range(N_CORES):
        y[c // 4] += res.results[c]["y"]
    return y, res


def kernel(**inputs) -> np.ndarray:
    y, _ = run(inputs, trace=False)
    return y



# revision 4
# speedup vs baseline: 2.1461x; 2.1461x over previous
"""Multi-head attention (RoPE) Trainium2 kernel, 8-way sharded.

Sharding: core c handles batch b = c//4 and 4 heads h0 = 4*(c%4).

Per-core program (v2 — pipelined around the ScalarE exp floor):
  Heads are processed as two pairs (0,1) and (2,3). Post-rope q/k live in
  pair tiles [128, 2048] with the even head in partitions 0-63 and the odd
  head in 64-127, so the two heads' score matmuls (K=64) run CONCURRENTLY
  as PE row-tiles (0,0)/(64,0) — 2x score throughput.

  PSUM (8 banks): sp0/sp1 score tiles [128,1024] f32 (2+2 banks, one per
  head, ping-ponged by the exp consumer), pv0/pv1 [65,512] accumulators
  (1+1), and two 1-bank projection chains (A/B) through which qkv pieces,
  v pieces and out-proj pieces flow as fillers under the exp-bound
  attention pipeline.

  attention loop (pair, qq in 4 x 512-token column quarters, ktpair in 8):
    S^T[kt,q] row-tiled pair -> exp(0.125 S) f16 (2 ACT calls, FD=1024)
    -> PV accumulate [V|1]^T @ P^T into pv (row 64 = softmax denominator)
  norm per (pair, qq): recip(denominator) -> gpsimd broadcast -> DVE mul
  out_proj per token block: y = attn_out^T @ w_out slices, DVE evac, DMA.

  host: y[b] = sum of the 4 per-core partials (fp32).
"""

import numpy as np

B = 2
N = 2048
C = 1024
HD = 64
HC = 4  # heads per core
N_CORES = 8
ROPE_BASE = 10000.0

_PROGRAM = None


def _rope_tables():
    inv_freq = 1.0 / (ROPE_BASE ** (np.arange(0, HD, 2, dtype=np.float32) / HD))
    t = np.arange(N, dtype=np.float32)
    freqs = np.einsum("i,j->ij", t, inv_freq).astype(np.float32)  # [N, 32]
    emb = np.concatenate([freqs, freqs], axis=-1)  # [N, 64]
    cos = np.cos(emb).astype(np.float32)
    sin = np.sin(emb).astype(np.float32)
    cosT = np.ascontiguousarray(np.tile(cos.T, (2, 1)))  # [128, 2048]
    sinT = sin.T.copy()  # [64, 2048]
    sinT_signed = np.concatenate([-sinT[:32], sinT[32:]], axis=0)
    sinT2 = np.ascontiguousarray(np.tile(sinT_signed, (2, 1)))  # [128, 2048]
    return cosT, sinT2


def _build_program():
    import concourse.mybir as mybir
    import concourse.tile as tile
    from concourse import bacc

    f32 = mybir.dt.float32
    f16 = mybir.dt.float16
    MUL = mybir.AluOpType.mult
    ADD = mybir.AluOpType.add
    EXP = mybir.ActivationFunctionType.Exp

    nc = bacc.Bacc("TRN2", target_bir_lowering=False, debug=False, num_devices=N_CORES)

    xT_d = nc.dram_tensor("xT", [C, N], f16, kind="ExternalInput").ap()
    wqk_d = nc.dram_tensor("wqkT", [C, 512], f16, kind="ExternalInput").ap()
    wv_d = nc.dram_tensor("wvT", [C, 256], f16, kind="ExternalInput").ap()
    wo_d = nc.dram_tensor("woT", [256, C], f16, kind="ExternalInput").ap()
    cos_d = nc.dram_tensor("cosT", [128, N], f32, kind="ExternalInput").ap()
    sin_d = nc.dram_tensor("sinT", [128, N], f32, kind="ExternalInput").ap()
    y_d = nc.dram_tensor("y", [N, C], f32, kind="ExternalOutput").ap()

    with tile.TileContext(nc) as tc:
        with (
            tc.tile_pool(name="persist", bufs=1) as persist,
            tc.tile_pool(name="work", bufs=2) as work,
            tc.tile_pool(name="psum", bufs=1, space="PSUM") as psp,
        ):
            # ---------------- persistent SBUF ----------------
            xT = persist.tile([128, 8, N], f16, tag="xT", name="xT")
            wqk = persist.tile([128, 8, 512], f16, tag="wqk", name="wqk")
            wv = persist.tile([128, 8, 256], f16, tag="wv", name="wv")
            wo = persist.tile([128, 2, C], f16, tag="wo", name="wo")
            cosT = persist.tile([128, N], f32, tag="cosT", name="cosT")
            sinT = persist.tile([128, N], f32, tag="sinT", name="sinT")
            # q-pair0, q-pair1, k-pair0, k-pair1  (matches wqkT col blocks)
            qk = [
                persist.tile([128, N], f16, tag=f"qk{t}", name=f"qk{t}")
                for t in range(4)
            ]
            vv = persist.tile([128, 16, HC, HD + 1], f16, tag="vv", name="vv")
            ao = [
                persist.tile([128, N], f16, tag=f"ao{p}", name=f"ao{p}")
                for p in range(2)
            ]

            # ---------------- helpers ----------------
            def dma_in():
                # order matters: earliest-needed first
                for ct in range(8):
                    nc.sync.dma_start(
                        wqk[:, ct, :], wqk_d[ct * 128 : (ct + 1) * 128, :]
                    )
                for ct in range(8):
                    nc.sync.dma_start(
                        xT[:, ct, 0:512], xT_d[ct * 128 : (ct + 1) * 128, 0:512]
                    )
                nc.sync.dma_start(cosT[:], cos_d[:, :])
                nc.sync.dma_start(sinT[:], sin_d[:, :])
                for ct in range(8):
                    nc.sync.dma_start(
                        xT[:, ct, 512:1024], xT_d[ct * 128 : (ct + 1) * 128, 512:1024]
                    )
                for ct in range(8):
                    nc.sync.dma_start(
                        wv[:, ct, :], wv_d[ct * 128 : (ct + 1) * 128, :]
                    )
                for pc in (2, 3):
                    sl = slice(pc * 512, (pc + 1) * 512)
                    for ct in range(8):
                        nc.sync.dma_start(xT[:, ct, sl], xT_d[ct * 128 : (ct + 1) * 128, sl])
                for i in range(2):
                    nc.sync.dma_start(wo[:, i, :], wo_d[i * 128 : (i + 1) * 128, :])

            def act_table_preload():
                scratch = work.tile([128, 16], f32, tag="dmy", name="dmy")
                nc.vector.memset(scratch[:], 0.0)
                dmye = work.tile([128, 16], f16, tag="dmye", name="dmye")
                nc.scalar.activation(dmye[:], scratch[:], EXP)

            def qk_piece(t, pc, chain):
                """project + rope one 512-token piece of qk tile t."""
                sl = slice(pc * 512, (pc + 1) * 512)
                bp = psp.tile([128, 512], f32, tag=chain, name=f"bp{t}_{pc}")
                for ct in range(8):
                    nc.tensor.matmul(
                        bp[:],
                        wqk[:, ct, t * 128 : (t + 1) * 128],
                        xT[:, ct, sl],
                        start=(ct == 0),
                        stop=(ct == 7),
                    )
                t_sb = work.tile([128, 512], f32, tag="ropet", name="rt")
                u_sb = work.tile([128, 512], f32, tag="ropeu", name="ru")
                nc.vector.tensor_tensor(t_sb[:], bp[:], cosT[:, sl], MUL)
                for o_lo, i_lo in [(0, 32), (32, 0), (64, 96), (96, 64)]:
                    nc.vector.tensor_tensor(
                        u_sb[o_lo : o_lo + 32, :],
                        bp[i_lo : i_lo + 32, :],
                        sinT[o_lo : o_lo + 32, sl],
                        MUL,
                    )
                nc.vector.tensor_tensor(qk[t][:, sl], t_sb[:], u_sb[:], ADD)

            def v_piece(tt, chain):
                """V' tile for one 128-token block (token-major)."""
                vp = psp.tile([128, 256], f32, tag=chain, name=f"vp{tt}")
                for ct in range(8):
                    nc.tensor.matmul(
                        vp[:],
                        xT[:, ct, tt * 128 : (tt + 1) * 128],
                        wv[:, ct, :],
                        start=(ct == 0),
                        stop=(ct == 7),
                    )
                nc.vector.tensor_copy(
                    vv[:, tt, :, 0:HD],
                    vp[:].rearrange("p (h d) -> p h d", h=HC),
                )

            def y_piece(tt, oc, chain):
                osl = slice(oc * 512, (oc + 1) * 512)
                yps = psp.tile([128, 512], f32, tag=chain, name=f"yps{tt}_{oc}")
                for p in range(2):
                    nc.tensor.matmul(
                        yps[:],
                        ao[p][:, tt * 128 : (tt + 1) * 128],
                        wo[:, p, osl],
                        start=(p == 0),
                        stop=(p == 1),
                    )
                ysb = work.tile([128, 512], f32, tag="ysb", bufs=3, name="ysb")
                nc.vector.tensor_copy(ysb[:], yps[:])
                nc.sync.dma_start(y_d[tt * 128 : (tt + 1) * 128, osl], ysb[:])

            # filler schedule: maps (pair, qq, i) -> list of thunks
            fillers = {}

            def add_fill(pair, qq, i, fn):
                fillers.setdefault((pair, qq, i), []).append(fn)

            # pair0 window: remaining v pieces, rest of qk0, all of qk1
            add_fill(0, 0, 0, lambda: qk_piece(2, 2, "pB"))
            for i in range(4):
                add_fill(0, 0, i, (lambda j: lambda: v_piece(8 + j, "pA"))(2 * i))
                add_fill(0, 0, i, (lambda j: lambda: v_piece(9 + j, "pA"))(2 * i))
            add_fill(0, 0, 2, lambda: qk_piece(2, 3, "pB"))
            add_fill(0, 0, 4, lambda: qk_piece(0, 1, "pA"))
            add_fill(0, 1, 0, lambda: qk_piece(3, 0, "pB"))
            add_fill(0, 1, 1, lambda: qk_piece(1, 0, "pA"))
            add_fill(0, 1, 2, lambda: qk_piece(3, 1, "pB"))
            add_fill(0, 1, 4, lambda: qk_piece(0, 2, "pA"))
            add_fill(0, 1, 5, lambda: qk_piece(3, 2, "pB"))
            add_fill(0, 2, 0, lambda: qk_piece(3, 3, "pB"))
            add_fill(0, 2, 1, lambda: qk_piece(1, 1, "pA"))
            add_fill(0, 2, 2, lambda: qk_piece(0, 3, "pA"))
            add_fill(0, 2, 3, lambda: qk_piece(1, 2, "pA"))
            add_fill(0, 3, 0, lambda: qk_piece(1, 3, "pA"))
            # pair1 window: out-proj pieces for the previous qq's token blocks
            for qq in range(1, 4):
                for j in range(4):
                    tt = (qq - 1) * 4 + j
                    add_fill(1, qq, j, (lambda t: lambda: y_piece(t, 0, "pA"))(tt))
                    add_fill(1, qq, j, (lambda t: lambda: y_piece(t, 1, "pB"))(tt))

            def attention_pair(pair):
                qt = qk[pair]
                kt_ = qk[2 + pair]
                for qq in range(4):
                    qsl = slice(qq * 512, (qq + 1) * 512)
                    pv0 = psp.tile([HD + 1, 512], f32, tag="pv0", name=f"pv0_{pair}_{qq}")
                    pv1 = psp.tile([HD + 1, 512], f32, tag="pv1", name=f"pv1_{pair}_{qq}")
                    for i in range(8):
                        ktA, ktB = 2 * i, 2 * i + 1
                        sp0 = psp.tile([128, 1024], f32, tag="sp0", name=f"sp0_{pair}_{qq}_{i}")
                        sp1 = psp.tile([128, 1024], f32, tag="sp1", name=f"sp1_{pair}_{qq}_{i}")
                        for half, kk in ((0, ktA), (1, ktB)):
                            ksl = slice(kk * 128, (kk + 1) * 128)
                            ssl = slice(half * 512, (half + 1) * 512)
                            nc.tensor.matmul(
                                sp0[:, ssl], kt_[0:64, ksl], qt[0:64, qsl],
                                start=True, stop=True,
                            )
                            nc.tensor.matmul(
                                sp1[:, ssl], kt_[64:128, ksl], qt[64:128, qsl],
                                start=True, stop=True,
                            )
                        es0 = work.tile([128, 1024], f16, tag="es0", bufs=3, name="es0")
                        es1 = work.tile([128, 1024], f16, tag="es1", bufs=3, name="es1")
                        nc.scalar.activation(es0[:], sp0[:], EXP, scale=float(HD**-0.5))
                        nc.scalar.activation(es1[:], sp1[:], EXP, scale=float(HD**-0.5))
                        for half, kk in ((0, ktA), (1, ktB)):
                            ssl = slice(half * 512, (half + 1) * 512)
                            nc.tensor.matmul(
                                pv0[:], vv[:, kk, 2 * pair, :], es0[:, ssl],
                                start=(i == 0 and half == 0),
                                stop=(i == 7 and half == 1),
                            )
                            nc.tensor.matmul(
                                pv1[:], vv[:, kk, 2 * pair + 1, :], es1[:, ssl],
                                start=(i == 0 and half == 0),
                                stop=(i == 7 and half == 1),
                            )
                        for fn in fillers.get((pair, qq, i), ()):
                            fn()
                    # normalize: ao[pair][j*64:(j+1)*64, qsl] = pv_j[0:64] / pv_j[64]
                    for j, pv in ((0, pv0), (1, pv1)):
                        rr = work.tile([1, 512], f32, tag="rr", name="rr")
                        nc.vector.tensor_copy(rr[:], pv[HD : HD + 1, :])
                        ra = work.tile([1, 512], f32, tag="ra", name="ra")
                        nc.vector.reciprocal_approx_fast(ra[:], rr[:])
                        nb = work.tile([64, 512], f32, tag="nb", name="nb")
                        nc.gpsimd.partition_broadcast(nb[:], ra[:])
                        nc.vector.tensor_tensor(
                            ao[pair][j * 64 : (j + 1) * 64, qsl],
                            pv[0:HD, :],
                            nb[:],
                            MUL,
                        )

            # ---------------- emission ----------------
            act_table_preload()
            dma_in()
            nc.vector.memset(vv[:, :, :, HD : HD + 1], 1.0)
            # boot: k-pair0 cols 0-1023, q-pair0 cols 0-511, v blocks 0-7
            qk_piece(2, 0, "pB")
            qk_piece(0, 0, "pA")
            qk_piece(2, 1, "pB")
            for tt in range(4):
                v_piece(tt, "sp0")
            for tt in range(4, 8):
                v_piece(tt, "sp1")

            attention_pair(0)
            attention_pair(1)

            # tail: out-proj for the last quarter's token blocks
            for j in range(4):
                tt = 12 + j
                y_piece(tt, 0, "pA")
                y_piece(tt, 1, "pB")

    nc.compile()
    return nc


def _get_program():
    global _PROGRAM
    if _PROGRAM is None:
        _PROGRAM = _build_program()
    return _PROGRAM


def _make_in_maps(x, w_qkv, w_out):
    x = np.asarray(x, dtype=np.float32)
    w_qkv = np.asarray(w_qkv, dtype=np.float32)
    w_out = np.asarray(w_out, dtype=np.float32)
    cosT, sinT = _rope_tables()
    in_maps = []
    for c in range(N_CORES):
        b = c // 4
        h0 = HC * (c % 4)
        rows = np.arange(h0 * HD, (h0 + HC) * HD)
        wq = w_qkv[rows]  # [256, 1024]
        wk = w_qkv[C + rows]
        wv = w_qkv[2 * C + rows]
        in_maps.append(
            {
                "xT": np.ascontiguousarray(x[b].T).astype(np.float16),
                "wqkT": np.ascontiguousarray(np.concatenate([wq, wk], 0).T).astype(
                    np.float16
                ),
                "wvT": np.ascontiguousarray(wv.T).astype(np.float16),
                "woT": np.ascontiguousarray(w_out[:, rows].T).astype(np.float16),
                "cosT": cosT,
                "sinT": sinT,
            }
        )
    return in_maps


def run(inputs, trace=False, trace_cores=None):
    from concourse.bass_utils import run_bass_kernel_spmd

    nc = _get_program()
    in_maps = _make_in_maps(inputs["x"], inputs["w_qkv"], inputs["w_out"])
    res = run_bass_kernel_spmd(
        nc,
        in_maps,
        core_ids=list(range(N_CORES)),
        trace=trace,
        trace_cores=trace_cores,
    )
    y = np.zeros((B, N, C), dtype=np.float32)
    for c in range(N_CORES):
        y[c // 4] += res.results[c]["y"]
    return y, res


def kernel(**inputs) -> np.ndarray:
    y, _ = run(inputs, trace=False)
    return y
